# revision 1
# baseline (speedup 1.0000x reference)
"""Trainium2 Bass kernel for nn_Block_82042465288934 (involution block).

Per-core layout: data-parallel over batch (B=8 images over 8 cores), one
image per core, channel-major [c=128 partitions, h*w=4096 free].

Pipeline per core:
  conv1+BN+ReLU folded on host -> PE matmul + ACT Relu -> t2 [32,4096] bf16
  involution weights: per-tap matmul with host-replicated conv2 rows
    (lhsT rows repeated x16 across group channels) -> PSUM holds w_rep
  ACT evacuates PSUM -> SBUF bf16 (fused +conv2 bias)
  DVE: 49-tap multiply-accumulate in bf16 (2x mode; two x_pad copies at
    byte offsets 0/+1 keep every tap 4B-aligned)
  LN: PE ones-matmul channel sums -> DMA reshape [1,8192]->[128,64] ->
    stats math -> DMA back -> K=1 broadcast matmuls -> DVE normalize
  MLP: PE matmuls (LN affine folded into pw1 on host), ACT erf-Gelu,
    residual + pw2 bias via one fused scalar_tensor_tensor.
"""

import numpy as np
import ml_dtypes

B, DIM, H, W = 8, 128, 64, 64
K = 7
PAD = 3
GC = 16
G = 8
RED = 4
HID = DIM // RED          # 32
N = H * W                 # 4096
NT = K * K                # 49 taps
HP = H + 2 * PAD          # 70 (padded row stride)
BN_EPS = 1e-5
LN_EPS = 1e-6
F2 = 2 * DIM              # 256
W32C = 213                # packed f32 weight blob columns
W16C = NT * DIM + 1 + 2 * F2  # packed bf16 weight blob columns

_BUILD_CACHE = {}

bf16 = ml_dtypes.bfloat16


def _build():
    """Trace + compile the single-core bass kernel. Cached per process."""
    if "nc" in _BUILD_CACHE:
        return _BUILD_CACHE["nc"]

    import concourse.bacc as bacc
    import concourse.tile as tile
    from concourse import mybir

    f32 = mybir.dt.float32
    b16 = mybir.dt.bfloat16
    AF = mybir.ActivationFunctionType
    OP = mybir.AluOpType

    nc = bacc.Bacc("TRN2", target_bir_lowering=False, debug=False, num_devices=1)

    # ---- DRAM I/O ----
    x_d = nc.dram_tensor("x", (DIM, N), f32, kind="ExternalInput")
    w32_d = nc.dram_tensor("w32", (DIM, W32C), f32, kind="ExternalInput")
    w16_d = nc.dram_tensor("w16", (DIM, W16C), b16, kind="ExternalInput")
    out_d = nc.dram_tensor("out", (DIM, N), f32, kind="ExternalOutput")

    with tile.TileContext(nc) as tc:
        with (
            tc.tile_pool(name="const", bufs=1) as const,
            tc.tile_pool(name="work", bufs=4) as work,
            tc.tile_pool(name="workg", bufs=6) as workg,
            tc.tile_pool(name="work2", bufs=2) as work2,
            tc.tile_pool(name="psum", bufs=2, space="PSUM") as psum,
        ):
            # ---- load inputs (x halves + 2 packed weight blobs, spread
            #      over two HWDGE queues so nothing serializes) ----
            x_sb = const.tile([DIM, N], f32)
            for half in range(2):
                hs = slice(half * 2048, (half + 1) * 2048)
                nc.sync.dma_start(out=x_sb[:, hs], in_=x_d.ap()[:, hs])
            w32_sb = const.tile([DIM, W32C], f32)
            nc.scalar.dma_start(out=w32_sb[:], in_=w32_d.ap())
            w16_sb = const.tile([DIM, W16C], b16)
            nc.scalar.dma_start(out=w16_sb[:], in_=w16_d.ap())
            w1T_sb = w32_sb[:, 0:HID]
            b1_sb = w32_sb[0:HID, HID : HID + 1]
            c2b_sb = w32_sb[:, 33:82]
            onesr_sb = w32_sb[0:1, 82:210]
            b1p_sb = w32_sb[:, 210:212]
            b2_sb = w32_sb[:, 212:213]
            c2wT_sb = w16_sb[0:HID, 0 : NT * DIM]
            onesc_sb = w16_sb[:, NT * DIM : NT * DIM + 1]
            w1pT_sb = w16_sb[:, NT * DIM + 1 : NT * DIM + 1 + F2]
            w2T_sb = w16_sb[:, NT * DIM + 1 + F2 : NT * DIM + 1 + 2 * F2]

            # preload ACT function tables while DMAs are in flight
            dummy = const.tile([DIM, 1], f32)
            nc.vector.memset(dummy[:], 0.0)
            dscr = const.tile([DIM, 1], f32)
            nc.scalar.activation(out=dscr[:], in_=dummy[:], func=AF.Gelu,
                                 bias=dummy[:])
            nc.scalar.activation(out=dscr[:], in_=dummy[:], func=AF.Relu,
                                 bias=dummy[:])
            nc.scalar.activation(out=dscr[:], in_=dummy[:], func=AF.Square,
                                 bias=dummy[:])
            nc.scalar.activation(out=dscr[:], in_=dummy[:], func=AF.Identity,
                                 bias=dummy[:])

            # ---- padded bf16 copies of x (offset 0 and +1 element for
            #      4B alignment of every tap). Memsets on DVE (free while the
            #      x DMA is in flight); interior casts on GPSIMD per half.
            xp0 = const.tile([DIM, HP * HP], b16)
            xp1 = const.tile([DIM, HP * HP + 2], b16)
            nc.vector.memset(xp0[:], 0.0)
            nc.vector.memset(xp1[:], 0.0)
            xp0v = xp0[:].rearrange("p (a b) -> p a b", a=HP, b=HP)
            xp1v = xp1[:, 1 : 1 + HP * HP].rearrange("p (a b) -> p a b", a=HP, b=HP)
            HHALF = H // 2  # 32 rows per half (2048 pixels)
            for half in range(2):
                xvh = x_sb[:, half * 2048 : (half + 1) * 2048].rearrange(
                    "p (a b) -> p a b", a=HHALF, b=W
                )
                r0 = PAD + half * HHALF
                nc.gpsimd.tensor_copy(
                    out=xp0v[:, r0 : r0 + HHALF, PAD : PAD + W], in_=xvh
                )
                nc.gpsimd.tensor_copy(
                    out=xp1v[:, r0 : r0 + HHALF, PAD : PAD + W], in_=xvh
                )

            # ---- conv1 + BN + ReLU -> t2 [HID, N] bf16 (2048-wide chunks) ----
            t2_sb = const.tile([HID, N], b16)
            for half in range(2):
                pc1 = psum.tile([HID, 2048], f32, tag="ps")
                for s in range(4):
                    nc.tensor.matmul(
                        out=pc1[:, s * 512 : (s + 1) * 512],
                        lhsT=w1T_sb,
                        rhs=x_sb[:, half * 2048 + s * 512 : half * 2048 + (s + 1) * 512],
                    )
                nc.scalar.activation(
                    out=t2_sb[:, half * 2048 : (half + 1) * 2048],
                    in_=pc1[:],
                    func=AF.Relu,
                    bias=b1_sb,
                )

            # ---- involution + LN + MLP, pipelined per pixel-row half ----
            # ~11 taps run their multiply-accumulate on GPSIMD (idle engine;
            # tensor_tensor never contends with DVE single-port 2x mode), the
            # rest on DVE; separate accumulators merged per half. Each half
            # finishes with its own LN + MLP + store so the tail of half 0
            # overlaps the tap loop of half 1.
            POOL_TAPS = {2, 7, 12, 17, 22, 27, 32, 37, 42, 47}
            acc_sb = const.tile([DIM, N], b16)
            accg_sb = const.tile([DIM, N], b16)
            accv = acc_sb[:].rearrange("p (a b) -> p a b", a=H, b=W)
            accgv = accg_sb[:].rearrange("p (a b) -> p a b", a=H, b=W)
            first_dve = min(t for t in range(NT) if t not in POOL_TAPS)
            first_gp = min(POOL_TAPS)
            y2_sb = accg_sb  # accg half is dead after merge; reuse as y^2
            yn_sb = const.tile([DIM, N], b16)
            out_sb = x_sb    # in-place residual: STT reads x, writes same chunk
            SJ = N // DIM    # 32 pixels per stats strip
            # stats_row layout: [half*4096 + p_local*64 + k*32 + j], k=0 sum,
            # k=1 sumsq on the way in; k=0 mu, k=1 rstd on the way back.
            stats_row = const.tile([1, 2 * N], f32)
            mr_row = stats_row
            stats_t = const.tile([DIM, 2 * SJ], f32)
            mr_t = const.tile([DIM, 2 * SJ], f32)
            zero_t = const.tile([DIM, 1], f32)
            nc.vector.memset(zero_t[:], 0.0)
            eps_t = const.tile([DIM, 1], f32)
            nc.vector.memset(eps_t[:], LN_EPS)

            def emit_taps(band, trange):
                px0, npx = band
                nrow = npx // W
                row0 = px0 // W
                for t in trange:
                    di, dj = t // K, t % K
                    # window offset (di+h)*70 + dj + w; odd dj uses the +1
                    # copy so the innermost run stays 4-byte aligned.
                    xsrc_v = xp0v if dj % 2 == 0 else xp1v
                    lhsT_t = c2wT_sb[:, t * DIM : (t + 1) * DIM]
                    pw = psum.tile([DIM, npx], f32, tag="ps")
                    for s in range(npx // 512):
                        n0 = px0 + s * 512
                        nc.tensor.matmul(
                            out=pw[:, s * 512 : (s + 1) * 512],
                            lhsT=lhsT_t,
                            rhs=t2_sb[:, n0 : n0 + 512],
                        )
                    if t in POOL_TAPS:
                        wrep = workg.tile([DIM, 2048], b16, tag="wrepg")
                    else:
                        wrep = work.tile([DIM, 2048], b16, tag="wrep")
                    wrep_ap = wrep[:, 0:npx]
                    nc.scalar.activation(
                        out=wrep_ap,
                        in_=pw[:],
                        func=AF.Identity,
                        bias=c2b_sb[:, t : t + 1],
                    )
                    wrepv = wrep_ap.rearrange("p (a b) -> p a b", a=nrow, b=W)
                    r0 = di + row0
                    xs = xsrc_v[:, r0 : r0 + nrow, dj : dj + W]
                    if t in POOL_TAPS:
                        eng, acv, first, ptag = nc.gpsimd, accgv, first_gp, "prodg"
                    else:
                        eng, acv, first, ptag = nc.vector, accv, first_dve, "prod"
                    av = acv[:, row0 : row0 + nrow, :]
                    if t == first:
                        eng.tensor_mul(av, wrepv, xs)
                    else:
                        prod = work2.tile([DIM, 32, W], b16, tag=ptag)
                        pv = prod[:, 0:nrow, :]
                        eng.tensor_mul(pv, wrepv, xs)
                        eng.tensor_add(av, av, pv)

            def emit_tail_a(band):
                """merge + y^2 + channel sums + stats math (through mr DMA)."""
                px0, npx = band
                hsl = slice(px0, px0 + npx)
                nst = npx // SJ  # stats strips in this band
                nc.gpsimd.tensor_add(acc_sb[:, hsl], acc_sb[:, hsl], accg_sb[:, hsl])
                nc.scalar.activation(
                    out=y2_sb[:, hsl], in_=acc_sb[:, hsl], func=AF.Square,
                    bias=zero_t[:],
                )
                ps1 = psum.tile([1, npx], f32, tag="ps")
                ps2 = psum.tile([1, npx], f32, tag="ps")
                for s in range(npx // 512):
                    n0 = px0 + s * 512
                    nc.tensor.matmul(
                        out=ps1[:, s * 512 : (s + 1) * 512],
                        lhsT=onesc_sb,
                        rhs=acc_sb[:, n0 : n0 + 512],
                    )
                    nc.tensor.matmul(
                        out=ps2[:, s * 512 : (s + 1) * 512],
                        lhsT=onesc_sb,
                        rhs=y2_sb[:, n0 : n0 + 512],
                    )
                srow_v = stats_row[
                    :, 2 * px0 : 2 * (px0 + npx)
                ].rearrange("o (p kj) -> o p kj", p=nst, kj=2 * SJ)
                nc.scalar.copy(
                    out=srow_v[:, :, 0:SJ],
                    in_=ps1[:].rearrange("o (p j) -> o p j", p=nst, j=SJ),
                )
                nc.scalar.copy(
                    out=srow_v[:, :, SJ : 2 * SJ],
                    in_=ps2[:].rearrange("o (p j) -> o p j", p=nst, j=SJ),
                )
                psl = slice(px0 // SJ, px0 // SJ + nst)
                nc.sync.dma_start(out=stats_t[psl, :], in_=srow_v)
                s1v = stats_t[psl, 0:SJ]
                s2v = stats_t[psl, SJ : 2 * SJ]
                mu_v = mr_t[psl, 0:SJ]
                nc.vector.tensor_scalar(
                    out=mu_v, in0=s1v, scalar1=1.0 / DIM, scalar2=None, op0=OP.mult
                )
                m2 = work2.tile([DIM, SJ], f32, tag="m2")
                nc.vector.tensor_mul(m2[psl, :], mu_v, mu_v)
                var = work2.tile([DIM, SJ], f32, tag="var")
                nc.vector.scalar_tensor_tensor(
                    out=var[psl, :], in0=s2v, scalar=1.0 / DIM, in1=m2[psl, :],
                    op0=OP.mult, op1=OP.subtract,
                )
                std = work2.tile([DIM, SJ], f32, tag="std")
                nc.scalar.activation(
                    out=std[psl, :], in_=var[psl, :], func=AF.Sqrt,
                    bias=eps_t[psl, :],
                )
                nc.vector.reciprocal(out=mr_t[psl, SJ : 2 * SJ], in_=std[psl, :])
                mrow_v = mr_row[
                    :, 2 * px0 : 2 * (px0 + npx)
                ].rearrange("o (p kj) -> o p kj", p=nst, kj=2 * SJ)
                nc.sync.dma_start(out=mrow_v, in_=mr_t[psl, :])

            def emit_tail_b(band):
                """broadcast + normalize + MLP + residual + store."""
                px0, npx = band
                hsl = slice(px0, px0 + npx)
                pmu = psum.tile([DIM, npx], f32, tag="ps")
                prs = psum.tile([DIM, npx], f32, tag="ps")
                for s in range(npx // 512):
                    base = 2 * px0 + s * 16 * 64
                    rhs_mu = stats_row[:, base : base + 1024].rearrange(
                        "o (p j) -> o p j", p=16, j=2 * SJ
                    )[:, :, 0:SJ]
                    rhs_rs = stats_row[:, base : base + 1024].rearrange(
                        "o (p j) -> o p j", p=16, j=2 * SJ
                    )[:, :, SJ : 2 * SJ]
                    nc.tensor.matmul(
                        out=pmu[:, s * 512 : (s + 1) * 512],
                        lhsT=onesr_sb,
                        rhs=rhs_mu,
                    )
                    nc.tensor.matmul(
                        out=prs[:, s * 512 : (s + 1) * 512],
                        lhsT=onesr_sb,
                        rhs=rhs_rs,
                    )
                yc = work2.tile([DIM, 2048], b16, tag="yc")
                nc.vector.tensor_sub(yc[:, 0:npx], acc_sb[:, hsl], pmu[:])
                nc.vector.tensor_mul(yn_sb[:, hsl], yc[:, 0:npx], prs[:])
                for s in range(max(1, npx // 1024)):
                    n0 = px0 + s * 1024
                    csz = min(1024, npx)
                    ph_a = psum.tile([DIM, 1024], f32, tag="ps")
                    ph_b = psum.tile([DIM, 1024], f32, tag="ps")
                    for q in range(csz // 512):
                        rhs = yn_sb[:, n0 + q * 512 : n0 + (q + 1) * 512]
                        nc.tensor.matmul(
                            out=ph_a[:, q * 512 : (q + 1) * 512],
                            lhsT=w1pT_sb[:, 0:DIM],
                            rhs=rhs,
                        )
                        nc.tensor.matmul(
                            out=ph_b[:, q * 512 : (q + 1) * 512],
                            lhsT=w1pT_sb[:, DIM:F2],
                            rhs=rhs,
                        )
                    ha = work2.tile([DIM, 1024], b16, tag="ha")
                    nc.scalar.activation(
                        out=ha[:, 0:csz], in_=ph_a[:, 0:csz], func=AF.Gelu,
                        bias=b1p_sb[:, 0:1],
                    )
                    hb = work2.tile([DIM, 1024], b16, tag="hb")
                    nc.scalar.activation(
                        out=hb[:, 0:csz], in_=ph_b[:, 0:csz], func=AF.Gelu,
                        bias=b1p_sb[:, 1:2],
                    )
                    po = psum.tile([DIM, 1024], f32, tag="ps")
                    for q in range(csz // 512):
                        sl = slice(q * 512, (q + 1) * 512)
                        nc.tensor.matmul(
                            out=po[:, sl], lhsT=w2T_sb[:, 0:DIM], rhs=ha[:, sl],
                            start=True, stop=False,
                        )
                        nc.tensor.matmul(
                            out=po[:, sl], lhsT=w2T_sb[:, DIM:F2], rhs=hb[:, sl],
                            start=False, stop=True,
                        )
                    nc.vector.scalar_tensor_tensor(
                        out=out_sb[:, n0 : n0 + csz],
                        in0=po[:, 0:csz],
                        scalar=b2_sb,
                        in1=x_sb[:, n0 : n0 + csz],
                        op0=OP.add,
                        op1=OP.add,
                    )
                nc.sync.dma_start(out=out_d.ap()[:, hsl], in_=out_sb[:, hsl])

            # three uneven bands: big first band, small last band so the
            # final (unavoidably serial) LN/MLP tail is short. Tail stages of
            # band i are woven into band i+1's tap stream.
            B0 = (0, 2048)
            B1 = (2048, 1024)
            B2 = (3072, 1024)
            emit_taps(B0, range(NT))
            emit_taps(B1, range(0, 9))
            emit_tail_a(B0)
            emit_taps(B1, range(9, 20))
            emit_tail_b(B0)
            emit_taps(B1, range(20, NT))
            emit_taps(B2, range(0, 9))
            emit_tail_a(B1)
            emit_taps(B2, range(9, 20))
            emit_tail_b(B1)
            emit_taps(B2, range(20, NT))
            emit_tail_a(B2)
            emit_tail_b(B2)

    nc.compile()
    _BUILD_CACHE["nc"] = nc
    return nc


def _prep_weights(inputs):
    """Host-side folding/packing of all weight tensors (shared by all cores)."""
    f = lambda k: np.asarray(inputs[k], dtype=np.float32)
    conv1_w, conv1_b = f("conv1_w"), f("conv1_b")
    bn_g, bn_b = f("bn_g"), f("bn_b")
    bn_mean, bn_var = f("bn_mean"), f("bn_var")
    conv2_w, conv2_b = f("conv2_w"), f("conv2_b")
    ln_g, ln_b = f("ln_g"), f("ln_b")
    pw1_w, pw1_b = f("pw1_w"), f("pw1_b")
    pw2_w, pw2_b = f("pw2_w"), f("pw2_b")

    s = bn_g / np.sqrt(bn_var + BN_EPS)
    w1f = conv1_w * s[:, None]
    b1f = conv1_b * s + (bn_b - bn_mean * s)

    gidx = np.arange(DIM) // GC  # group of each channel
    # c2wT[r, t*128 + c] = conv2_w[g(c)*49 + t, r]
    c2wT = np.empty((HID, NT * DIM), dtype=np.float32)
    for t in range(NT):
        c2wT[:, t * DIM : (t + 1) * DIM] = conv2_w[gidx * NT + t].T
    c2b_rep = conv2_b[gidx[:, None] * NT + np.arange(NT)[None, :]]  # [128, 49]

    W1p = pw1_w * ln_g[None, :]
    b1p = pw1_b + pw1_w @ ln_b
    b1p2 = np.stack([b1p[:DIM], b1p[DIM:]], axis=1)  # [128, 2]
    # w2T[p, k*128 + c] = pw2_w[c, k*128 + p]
    w2T = np.empty((DIM, F2), dtype=np.float32)
    w2T[:, 0:DIM] = pw2_w.T[0:DIM]
    w2T[:, DIM:F2] = pw2_w.T[DIM:F2]

    W32C = 213
    w32 = np.zeros((DIM, W32C), dtype=np.float32)
    w32[:, 0:HID] = w1f.T
    w32[0:HID, HID] = b1f
    w32[:, 33:82] = c2b_rep
    w32[0, 82:210] = 1.0          # onesr
    w32[:, 210:212] = b1p2
    w32[:, 212] = pw2_b
    W16C = NT * DIM + 1 + 2 * F2
    w16 = np.zeros((DIM, W16C), dtype=np.float32)
    w16[0:HID, 0 : NT * DIM] = c2wT
    w16[:, NT * DIM] = 1.0        # onesc
    w16[:, NT * DIM + 1 : NT * DIM + 1 + F2] = W1p.T
    w16[:, NT * DIM + 1 + F2 :] = w2T
    return {
        "w32": w32,
        "w16": w16.astype(bf16),
    }


def _get_runner(nc, n_cores):
    """Build (once) the jitted SPMD executable + metadata for running the
    bass module on `n_cores` devices with pre-sharded inputs (avoids XLA
    data-movement modules that the generic neuronx-cc path here cannot
    compile)."""
    if "runner" in _BUILD_CACHE:
        return _BUILD_CACHE["runner"]

    import jax
    from jax.sharding import Mesh, NamedSharding, PartitionSpec
    from jax.experimental.shard_map import shard_map
    from concourse import bass2jax, mybir

    bass2jax.install_neuronx_cc_hook()

    in_names, out_names, out_avals, zero_outs = [], [], [], []
    for alloc in nc.m.functions[0].allocations:
        if not isinstance(alloc, mybir.MemoryLocationSet):
            continue
        name = alloc.memorylocations[0].name
        if alloc.kind == "ExternalInput":
            in_names.append(name)
        elif alloc.kind == "ExternalOutput":
            shape = tuple(alloc.tensor_shape)
            dtype = mybir.dt.np(alloc.dtype)
            out_names.append(name)
            out_avals.append(jax.core.ShapedArray(shape, dtype))
            zero_outs.append(np.zeros(shape, dtype))
    n_params = len(in_names)
    n_outs = len(out_avals)
    all_names = in_names + out_names
    donate = tuple(range(n_params, n_params + n_outs))

    def _body(*args):
        outs = bass2jax._bass_exec_p.bind(
            *args,
            out_avals=tuple(out_avals),
            in_names=tuple(all_names),
            out_names=tuple(out_names),
            lowering_input_output_aliases=(),
            sim_require_finite=True,
            sim_require_nnan=True,
            nc=nc,
        )
        return tuple(outs)

    devices = jax.devices()[:n_cores]
    mesh = Mesh(np.asarray(devices), ("core",))
    in_specs = (PartitionSpec("core"),) * (n_params + n_outs)
    out_specs = (PartitionSpec("core"),) * n_outs
    sharded = jax.jit(
        shard_map(
            _body, mesh=mesh, in_specs=in_specs, out_specs=out_specs, check_rep=False
        ),
        donate_argnums=donate,
        keep_unused=True,
    )

    def make_global(per_core_arrays):
        shards = [
            jax.device_put(np.ascontiguousarray(a), d)
            for a, d in zip(per_core_arrays, devices)
        ]
        shape = (n_cores * shards[0].shape[0],) + tuple(shards[0].shape[1:])
        sharding = NamedSharding(mesh, PartitionSpec("core"))
        return jax.make_array_from_single_device_arrays(shape, sharding, shards)

    # partition_id is auto-declared by bass; feed each core its index.
    pid_name = nc.partition_id_tensor.name if nc.partition_id_tensor else None
    pid_shape, pid_dtype = None, None
    if pid_name is not None:
        for alloc in nc.m.functions[0].allocations:
            if (
                isinstance(alloc, mybir.MemoryLocationSet)
                and alloc.memorylocations[0].name == pid_name
            ):
                pid_shape = tuple(alloc.tensor_shape)
                pid_dtype = mybir.dt.np(alloc.dtype)

    runner = {
        "sharded": sharded,
        "make_global": make_global,
        "in_names": in_names,
        "out_names": out_names,
        "out_avals": out_avals,
        "zero_outs": zero_outs,
        "n_cores": n_cores,
        "pid": (pid_name, pid_shape, pid_dtype),
    }
    _BUILD_CACHE["runner"] = runner
    return runner


def _run_spmd(nc, in_maps):
    r = _get_runner(nc, len(in_maps))
    n_cores = r["n_cores"]
    pid_name, pid_shape, pid_dtype = r["pid"]
    if pid_name is not None:
        for c, m in enumerate(in_maps):
            m[pid_name] = np.full(pid_shape, c, dtype=pid_dtype)
    make_global = r["make_global"]
    args = [make_global([m[name] for m in in_maps]) for name in r["in_names"]]
    args += [make_global([z] * n_cores) for z in r["zero_outs"]]
    out_arrs = r["sharded"](*args)
    results = []
    for c in range(n_cores):
        results.append(
            {
                name: np.asarray(out_arrs[i].addressable_shards[c].data)
                for i, name in enumerate(r["out_names"])
            }
        )
    return results


def kernel(**inputs) -> np.ndarray:
    nc = _build()
    weights = _prep_weights(inputs)
    x = np.asarray(inputs["x"], dtype=np.float32).reshape(B, DIM, N)

    in_maps = []
    for b in range(B):
        m = dict(weights)
        m["x"] = np.ascontiguousarray(x[b])
        in_maps.append(m)

    results = _run_spmd(nc, in_maps)
    out = np.stack([r["out"] for r in results]).reshape(B, DIM, H, W)
    return out.astype(np.float32)


if __name__ == "__main__":
    _build()
    print("build ok")

